# revision 20
# baseline (speedup 1.0000x reference)
"""Position attention module (spatial self-attention) for Trainium2.

Shapes (hardcoded): x [8, 512, 64, 64], inter_channels CI=64, N=H*W=4096.
Sharding: data-parallel over batch B=8 across the 8 NeuronCores (one batch
element per core); 1x1-conv weights replicated.

Per-core algorithm (all in f32):
  theta/phi computed fused as PT1=[theta;phi] and PT2=[phi;theta] (two
  [128, N] tiles) so that both operands of the QK^T matmul are available at
  SBUF partitions 0-63 AND 64-127 (enables PE row-group packing of the
  K=64 energy matmuls).
  Energy is computed TRANSPOSED: E^T[m, n] = sum_c phi[c,m] theta[c,n],
  so exp(E^T) is directly the moving operand of the PV matmul (no PE
  transposes anywhere). The softmax denominator comes for free from a
  ones-column appended to g^T (row 64 of the PV accumulator).
  Normalization multiplies by reciprocal row-sums broadcast across
  partitions with a K=1 matmul. The output conv has the BN scale folded
  into its weights; the residual +x is folded in as an identity matmul
  accumulating into the same PSUM bank as the out-conv.
"""

import numpy as np

import concourse.bass as bass
import concourse.mybir as mybir
import concourse.tile as tile
from bass_rust import add_dep_helper
from concourse import bacc
from concourse.bass_utils import run_bass_kernel_spmd

F32 = mybir.dt.float32
EXP = mybir.ActivationFunctionType.Exp

B = 8
C = 512
CI = 64
H = W = 64
N = H * W          # 4096
KC = C // 128      # 4 contraction chunks over C
MC = N // 128      # 32 key chunks
NBLK = 1024        # query-block width (one pass)
NPASS = N // NBLK  # 4
BN_EPS = 1e-5

_PROGRAM_CACHE: dict = {}


def _build_program():
    nc = bacc.Bacc("TRN2", target_bir_lowering=False, debug=False)

    d_x = nc.dram_tensor("x", [C, N], F32, kind="ExternalInput").ap()
    # [128, KC*128]: column block k holds rows [k*128,(k+1)*128) of
    # concat([w_theta.T, w_phi.T], 1) (resp. [w_phi.T, w_theta.T] for w21)
    d_w12 = nc.dram_tensor("w12", [128, KC * 128], F32, kind="ExternalInput").ap()
    d_w21 = nc.dram_tensor("w21", [128, KC * 128], F32, kind="ExternalInput").ap()
    d_b12 = nc.dram_tensor("b12", [128, 1], F32, kind="ExternalInput").ap()
    d_b21 = nc.dram_tensor("b21", [128, 1], F32, kind="ExternalInput").ap()
    d_wg = nc.dram_tensor("wg", [128, KC * CI], F32, kind="ExternalInput").ap()
    d_bg = nc.dram_tensor("bg", [1, CI], F32, kind="ExternalInput").ap()
    d_wo = nc.dram_tensor("wo", [CI, C], F32, kind="ExternalInput").ap()
    d_tb = nc.dram_tensor("tb", [128, KC], F32, kind="ExternalInput").ap()
    d_id = nc.dram_tensor("ident", [128, 128], F32, kind="ExternalInput").ap()
    d_ones = nc.dram_tensor("ones64", [1, CI], F32, kind="ExternalInput").ap()
    d_out = nc.dram_tensor("out", [C, N], F32, kind="ExternalOutput").ap()

    with tile.TileContext(nc) as tc:
        with (
            tc.tile_pool(name="consts", bufs=1) as consts,
            tc.tile_pool(name="xp", bufs=1) as xp,
            tc.tile_pool(name="ptp", bufs=1) as ptp,
            tc.tile_pool(name="gtp", bufs=1) as gtp,
            tc.tile_pool(name="attnp", bufs=3) as attnp,
            tc.tile_pool(name="ynp", bufs=2) as ynp,
            tc.tile_pool(name="outp", bufs=4) as outp,
            tc.tile_pool(name="recp", bufs=2) as recp,
            tc.tile_pool(name="bcp", bufs=2) as bcp,
            tc.tile_pool(name="psE", bufs=2, space="PSUM") as psE,
            tc.tile_pool(name="psY", bufs=1, space="PSUM") as psY,
            tc.tile_pool(name="psS", bufs=2, space="PSUM") as psS,
        ):
            dma_insts = []

            # ---- load constants ----
            w12 = consts.tile([128, KC * 128], F32)
            dma_insts.append(nc.sync.dma_start(out=w12, in_=d_w12))
            w21 = consts.tile([128, KC * 128], F32)
            dma_insts.append(nc.sync.dma_start(out=w21, in_=d_w21))
            b12 = consts.tile([128, 1], F32)
            nc.sync.dma_start(out=b12, in_=d_b12)
            b21 = consts.tile([128, 1], F32)
            nc.sync.dma_start(out=b21, in_=d_b21)
            wg = consts.tile([128, KC * CI], F32)
            dma_insts.append(nc.sync.dma_start(out=wg, in_=d_wg))
            wo = consts.tile([CI, C], F32)
            dma_insts.append(nc.sync.dma_start(out=wo, in_=d_wo))
            tb = consts.tile([128, KC], F32)
            nc.sync.dma_start(out=tb, in_=d_tb)
            ident = consts.tile([128, 128], F32)
            dma_insts.append(nc.sync.dma_start(out=ident, in_=d_id))
            ones64 = consts.tile([1, CI], F32)
            dma_insts.append(nc.sync.dma_start(out=ones64, in_=d_ones))
            # b_g broadcast across partitions: [1, CI] -> [128, CI]
            bgb = consts.tile([128, CI], F32)
            nc.sync.dma_start(
                out=bgb,
                in_=bass.AP(
                    tensor=d_bg.tensor, offset=d_bg.offset, ap=[[0, 128]] + d_bg.ap[1:]
                ),
            )

            # Wait-absorber: walrus gives fp32 fused LDWEIGHTS+MATMUL a single
            # sync-wait slot, and sequencer-only nops don't advance the PE
            # engine's observed semaphore clock. A dummy bf16 ldweights is a
            # real PE engine instruction with no architectural effect on
            # self-loading fp32 matmuls, so it can absorb waits.
            dmy = consts.tile([1, 1], mybir.dt.bfloat16)
            dmy_set = nc.vector.memset(dmy, 0.0)

            def pe_absorb(producers):
                # the S3_LW struct has a single wait slot, so each absorber
                # ldweights may carry at most one semaphore wait: emit one
                # per producer, chained in order
                prev = None
                for pr in producers:
                    ld = nc.tensor.ldweights(dmy)
                    add_dep_helper(ld.ins, pr.ins, reason="absorb waits on PE")
                    if prev is not None:
                        add_dep_helper(ld.ins, prev.ins, sync=False)
                    prev = ld
                return prev

            # primer: absorb the dmy-memset DVE tick by itself
            pe_absorb([dmy_set])

            # ---- load x: [512, N] as 4 chunks of 128 partitions ----
            x_sb = xp.tile([128, KC, N], F32)
            for k in range(KC):
                dma_insts.append(
                    nc.sync.dma_start(
                        out=x_sb[:, k, :], in_=d_x[k * 128 : (k + 1) * 128, :]
                    )
                )

            # fp32 fused LDWEIGHTS+MATMUL carries only ONE sync-wait slot in
            # walrus codegen. Absorb the DMA-queue semaphores into a PE nop so
            # the first matmuls need no DMA waits of their own.
            nop = pe_absorb(dma_insts)

            # ---- PT1 = [theta; phi], PT2 = [phi; theta]  ([128, N]) ----
            pt1 = ptp.tile([128, N], F32)
            pt2 = ptp.tile([128, N], F32)
            for w_sb, b_sb, pt in ((w12, b12, pt1), (w21, b21, pt2)):
                for ns in range(N // 512):
                    sl = slice(ns * 512, (ns + 1) * 512)
                    pt_ps = psS.tile([128, 512], F32, tag="scratch", name="pt_ps")
                    for k in range(KC):
                        mm = nc.tensor.matmul(
                            pt_ps,
                            lhsT=w_sb[:, k * 128 : (k + 1) * 128],
                            rhs=x_sb[:, k, sl],
                            start=(k == 0),
                            stop=(k == KC - 1),
                        )
                        if k == 0:
                            add_dep_helper(mm.ins, nop.ins, sync=False)
                    nc.vector.tensor_scalar_add(pt[:, sl], pt_ps, b_sb)

            # ---- gT augmented with ones column: [128, MC, 65] ----
            gt = gtp.tile([128, MC, 65], F32)
            nc.vector.memset(gt, 1.0)
            last_gt_add = None
            for mi in range(MC):
                g_ps = psS.tile([128, CI], F32, tag="scratch", name="g_ps")
                for k in range(KC):
                    mm = nc.tensor.matmul(
                        g_ps,
                        lhsT=x_sb[:, k, mi * 128 : (mi + 1) * 128],
                        rhs=wg[:, k * CI : (k + 1) * CI],
                        start=(k == 0),
                        stop=(k == KC - 1),
                    )
                    if k == 0:
                        add_dep_helper(mm.ins, nop.ins, sync=False)
                last_gt_add = nc.vector.tensor_add(gt[:, mi, 0:CI], g_ps, bgb)

            gt_nop = pe_absorb([last_gt_add])

            # ---- attention main loop ----
            prev_pass_insts = [gt_nop]
            for p in range(NPASS):
                # absorb cross-pass waits (prev epilogue DVE consumers of
                # y_ps, prev exp ACT ticks, prev PE psum-bank producers) on a
                # PE nop so the pass's first matmuls keep <=1 wait
                pass_nop = pe_absorb(prev_pass_insts)
                prev_pass_insts = []
                n0 = p * NBLK
                y_ps = psY.tile([CI + 1, NBLK], F32, tag="y", name="y_ps")
                for mi in range(MC):
                    msl = slice(mi * 128, (mi + 1) * 128)
                    e_ps = psE.tile([128, NBLK], F32, tag="energy", name="e_ps")
                    # E^T[m, n] = sum_c phi[c, m] * theta[c, n]
                    # even mi -> array rows 0-63, odd mi -> rows 64-127
                    # (PT1/PT2 hold both placements; row-group packing lets
                    # consecutive K=64 matmuls run concurrently on the PE)
                    if mi % 2 == 0:
                        lhsT, rhs_pt = pt2[0:64, msl], pt1
                        rl = slice(0, 64)
                    else:
                        lhsT, rhs_pt = pt1[64:128, msl], pt2
                        rl = slice(64, 128)
                    for s in range(NBLK // 512):
                        sl = slice(s * 512, (s + 1) * 512)
                        mm = nc.tensor.matmul(
                            e_ps[:, sl],
                            lhsT=lhsT,
                            rhs=rhs_pt[rl, n0 + s * 512 : n0 + (s + 1) * 512],
                            start=True,
                            stop=True,
                        )
                        if mi == 0 and s == 0:
                            add_dep_helper(mm.ins, pass_nop.ins, sync=False)
                    at = attnp.tile([128, NBLK], F32, tag="attn", name="at")
                    exp_i = nc.scalar.activation(out=at, in_=e_ps, func=EXP)
                    # absorb the exp ACT tick on a PE nop so PV keeps its
                    # single wait slot for the gt DVE dependency
                    mi_nop = pe_absorb([exp_i])
                    for s in range(NBLK // 512):
                        sl = slice(s * 512, (s + 1) * 512)
                        mm = nc.tensor.matmul(
                            y_ps[:, sl],
                            lhsT=gt[:, mi, :],
                            rhs=at[:, sl],
                            start=(mi == 0),
                            stop=(mi == MC - 1),
                        )
                        if s == 0:
                            add_dep_helper(mm.ins, mi_nop.ins, sync=False)
                        if mi == 0 and s == 0:
                            add_dep_helper(mm.ins, pass_nop.ins, sync=False)
                        if mi == MC - 1:
                            prev_pass_insts.append(mm)
                    if mi >= MC - 2:
                        prev_pass_insts.append(exp_i)

                # ---- normalize + out-conv + residual + bias ----
                rec = recp.tile([1, NBLK], F32, name="rec")
                nc.vector.reciprocal(rec, y_ps[CI : CI + 1, :])
                yn = ynp.tile([CI, NBLK], F32, name="yn")
                for s in range(NBLK // 512):
                    sl = slice(s * 512, (s + 1) * 512)
                    bc_ps = psS.tile([CI, 512], F32, tag="scratch", name="bc_ps")
                    nc.tensor.matmul(
                        bc_ps, lhsT=ones64, rhs=rec[:, sl], start=True, stop=True
                    )
                    bc_sb = bcp.tile([CI, 512], F32, name="bc_sb")
                    nc.vector.tensor_copy(bc_sb, bc_ps)
                    mul_i = nc.vector.tensor_mul(yn[:, sl], y_ps[0:CI, sl], bc_sb)
                    prev_pass_insts.append(mul_i)
                for s in range(NBLK // 512):
                    nsl = slice(n0 + s * 512, n0 + (s + 1) * 512)
                    for c in range(KC):
                        z_ps = psS.tile([128, 512], F32, tag="scratch", name="z_ps")
                        nc.tensor.matmul(
                            z_ps,
                            lhsT=wo[:, c * 128 : (c + 1) * 128],
                            rhs=yn[:, s * 512 : (s + 1) * 512],
                            start=True,
                            stop=False,
                        )
                        nc.tensor.matmul(
                            z_ps,
                            lhsT=ident,
                            rhs=x_sb[:, c, nsl],
                            start=False,
                            stop=True,
                        )
                        o_sb = outp.tile([128, 512], F32, name="o_sb")
                        nc.vector.tensor_scalar_add(o_sb, z_ps, tb[:, c : c + 1])
                        nc.sync.dma_start(
                            out=d_out[c * 128 : (c + 1) * 128, nsl], in_=o_sb
                        )

    # bacc legalizes waits (move_matmul_waits_to_ldweights +
    # generate_event_semaphores) for walrus's per-instruction wait limits
    nc.compile()

    # walrus fp32 fused-LDWEIGHTS matmuls support a single sync-wait slot;
    # verify nothing exceeds it post-compile (deterministic program, so
    # passing here means passing at run time).
    bad = []
    for blk in nc.m.functions[0].blocks:
        for inst in blk.instructions:
            if type(inst).__name__ in ("InstMatmult", "InstLdweights"):
                si = inst.sync_info
                if si is not None and len(si.on_wait) > 1:
                    bad.append(
                        (inst.name, [(w.ant_name, w.wait_value) for w in si.on_wait])
                    )
    if bad:
        raise RuntimeError(f"PE instructions with >1 sync wait: {bad}")
    return nc


def _prep_shared(inputs):
    f = lambda k: np.asarray(inputs[k], np.float32)
    w_theta, b_theta = f("w_theta"), f("b_theta")
    w_phi, b_phi = f("w_phi"), f("b_phi")
    w_g, b_g = f("w_g"), f("b_g")
    w_out, b_out = f("w_out"), f("b_out")
    gam = f("bn_gamma") / np.sqrt(f("bn_var") + BN_EPS)
    tbias = gam * b_out + f("bn_beta") - f("bn_mean") * gam  # [C]

    def chunked(a):  # [C, M] -> [128, KC*M] with column block k = rows of chunk k
        m = a.shape[1]
        return np.ascontiguousarray(
            a.reshape(KC, 128, m).transpose(1, 0, 2).reshape(128, KC * m)
        )

    w12t = np.concatenate([w_theta.T, w_phi.T], axis=1)  # [C, 128]
    w21t = np.concatenate([w_phi.T, w_theta.T], axis=1)
    return {
        "w12": chunked(w12t),
        "w21": chunked(w21t),
        "b12": np.concatenate([b_theta, b_phi])[:, None].astype(np.float32),
        "b21": np.concatenate([b_phi, b_theta])[:, None].astype(np.float32),
        "wg": chunked(w_g.T.copy()),
        "bg": np.ascontiguousarray(b_g[None, :]),
        "wo": np.ascontiguousarray((w_out * gam[:, None]).T),
        "tb": np.ascontiguousarray(tbias.reshape(KC, 128).T),
        "ident": np.eye(128, dtype=np.float32),
        "ones64": np.ones((1, CI), np.float32),
    }


def kernel(_trace=False, _tmpdir=None, **inputs):
    x = np.asarray(inputs["x"], np.float32).reshape(B, C, N)
    if "nc" not in _PROGRAM_CACHE:
        _PROGRAM_CACHE["nc"] = _build_program()
    nc = _PROGRAM_CACHE["nc"]

    shared = _prep_shared(inputs)
    in_maps = [dict(shared, x=np.ascontiguousarray(x[b])) for b in range(B)]
    res = run_bass_kernel_spmd(
        nc, in_maps, core_ids=list(range(B)), trace=_trace, tmpdir=_tmpdir
    )
    if _trace:
        _PROGRAM_CACHE["last_results"] = res
    out = np.stack([res.results[b]["out"] for b in range(B)])
    return out.reshape(B, C, H, W)
